# revision 14
# baseline (speedup 1.0000x reference)
"""EuclidConv + training-mode BatchNorm on 8 Trainium2 NeuronCores.

Math (reference): out = BN(2*conv(x,w) + conv(x^2, ones3x3) + ||w_f||^2),
BN over global-batch stats. The per-filter ||w||^2 term is channel-constant,
so BN's mean subtraction cancels it exactly -> never computed.

Sharding: HYBRID. core c -> (chgrp = c//4, bgrp = c%4): 128 of 256 output
channels x 8 of 32 images. This gives full-width M=128 matmuls (the pure
channel-sharded layout only fills 32 of 128 PE output columns), 4x less PE
streaming. The price: BN statistics must be reduced across the 4 bgrps that
share a channel group -> one tiny 4-rank AllGather of [128,2] partial
(sum, sumsq) + local fold.

Per image m (padded 30x30 grid, fp16):
  u_m = x_m^2                                      (img0 DVE, rest ACT)
  r4 psum = ones128.T @ u_m    (channel sums of x^2, replicated over all
                                128 partitions; 2 MMs)
  rc = r4 - 128*validmap       (DVE, fp16, centered for precision)
  box filter: vv = 3-tap vertical (DVE, stride-30 = pair-aligned 2x mode),
  te = vv[0]+vv[+2] (2x), tf = te + vv[+1] (GpSimd - odd offset would be
  1x-mode on DVE anyway, and DVE is the busier engine)
Conv accumulation group per (img, yt-half) [128,392] psum:
  identity.T @ tf_view         (start=True: seeds psum with t1 - 128*count)
  sum_k (2w)_k.T @ x_view      (9 offsets, full M=128)
  ones1.T @ cmap_view          (stop=True: re-adds 128*count)
Drain: ACT copy psum->s_sb with accum S; ACT square with accum Q.
Stats: fold S,Q over 8 local images -> [128,2]; 4-rank AllGather via HBM
bounce; fold 4 ranks; A = gamma*rsqrt(var+eps), B = beta - mean*A;
normalize out = s*A+B (DVE/ACT alternation, fp16) -> dual-queue DMA out.

Collective latency: the first collective of an execution pays a ~25-45us
NRT/ncfw entry barrier that runs on TOPSP concurrently with compute -- a
dummy AllGather triggered at kernel start absorbs it, so the real stats
AllGather starts ~1-2us after its trigger. Chunk-0 box chains are split
into y-halves and its conv is img-major so the PE pipeline fills ~8us
sooner; a memset-fed matmul warmup flips the PE HAM clock gate to 2.4GHz
during the input-DMA window.

Host-side prep is layout/sharding only: pad+transpose+cast of x, weight
transpose/scale, constant masks.
"""
import json

import numpy as np

import subprocess as _subprocess

import concourse.bass as bass
import concourse.bass_utils as _bass_utils
import concourse.mybir as mybir
import concourse.tile as tile
from concourse.ap import AP
from concourse.bass_utils import run_bass_kernel_spmd
from concourse.vector_clock import ScopedClock, VectorClock

F16 = mybir.dt.float16
F32 = mybir.dt.float32


class _WalrusLdwOpt:
    """Enable walrus's LDWEIGHTS dedup pass for this kernel's compiles.
    bass_utils hardcodes --enable-ldw-opt=false; this kernel issues runs of
    4+ matmuls sharing one stationary operand, where redundant per-matmul
    weight reloads serialize ~110ns each on the PE."""

    def __getattr__(self, name):
        return getattr(_subprocess, name)

    def check_call(self, argv, **kw):
        if (
            isinstance(argv, list)
            and argv
            and "walrus_driver" in str(argv[0])
        ):
            argv = [
                "--enable-ldw-opt=true" if a == "--enable-ldw-opt=false" else a
                for a in argv
            ]
        return _subprocess.check_call(argv, **kw)


_bass_utils.subprocess = _WalrusLdwOpt()

N_CORES = 8
NIMG_L = 8  # images per core
HP = 30
NPIX = HP * HP
NV = 28 * 28
NHW = 32 * NV  # global batch pixels per channel
EPS = 1e-5
CC_GROUPS = [[0, 1, 2, 3], [4, 5, 6, 7]]
N_WARM = 30

_split_ctr = [0]


def _split_waits_json(bir: bytes, max_waits: int = 1) -> bytes:
    """This container's walrus rejects instructions with >1 sync wait.
    Hoist excess waits onto EventSemaphore instructions inserted before the
    offender on the same engine stream."""
    m = json.loads(bir)
    for f in m["functions"]:
        for bb in f["blocks"]:
            newinsts = []
            for ins in bb["instructions"]:
                si = ins.get("sync_info")
                if si:
                    waits = si.get("on_wait") or []
                    if len(waits) > max_waits:
                        extra, keep = waits[:-max_waits], waits[-max_waits:]
                        for w_ in extra:
                            _split_ctr[0] += 1
                            newinsts.append(
                                {
                                    "debug": ins.get("debug", 0),
                                    "engine": ins["engine"],
                                    "ins": [],
                                    "outs": [],
                                    "name": f"antsplitw-{_split_ctr[0]}",
                                    "opcode": "EventSemaphore",
                                    "sync_info": {"on_update": [], "on_wait": [w_]},
                                }
                            )
                        si["on_wait"] = keep
                newinsts.append(ins)
            bb["instructions"] = newinsts
    return json.dumps(m).encode()


class _PatchedBass(bass.Bass):
    def to_json_bytes(self):
        return _split_waits_json(super().to_json_bytes())


class _SplitDrainTileContext(tile.TileContext):
    """Split the tile-exit drain's waits into single-wait drains (same
    walrus limitation as above)."""

    def _drain_and_barrier(self, tick_clock, wait_clock):
        g = tick_clock.global_clock
        n = len(g)
        for i in range(n):
            if g[i] > 0:
                vec = [0] * n
                vec[i] = g[i]
                d = self.nc.sync.drain()
                wait_clock.add_sem_waits(d.ins, ScopedClock({None: VectorClock(vec)}))
        self.nc.sync.drain()
        self.nc.all_engine_barrier()
        assert self.sems is not None
        popped = self.nc._tile_sem_poison_stack.pop()
        assert popped is self._sem_poison
        self.nc.clear_and_free_semaphores(list(self.sems.allocated().values()))
        self.nc.all_engine_barrier()


def _build_nc():
    nc = _PatchedBass(num_devices=N_CORES)
    xh = nc.dram_tensor("xh", [128, NIMG_L * NPIX], F16, kind="ExternalInput")
    wt = nc.dram_tensor("wt", [128, 9 * 128], F16, kind="ExternalInput")
    ones128d = nc.dram_tensor("ones128", [128, 128], F16, kind="ExternalInput")
    id128d = nc.dram_tensor("id128", [128, 128], F16, kind="ExternalInput")
    onesrd = nc.dram_tensor("onesr", [1, 128], F16, kind="ExternalInput")
    comp16d = nc.dram_tensor("comp16", [128, NPIX], F16, kind="ExternalInput")
    cmap16d = nc.dram_tensor("cmap16", [1, 840], F16, kind="ExternalInput")
    cst32d = nc.dram_tensor("cst32", [128, 3], F32, kind="ExternalInput")
    y = nc.dram_tensor("y", [NIMG_L, 128, 28, 28], F16, kind="ExternalOutput")

    with _SplitDrainTileContext(nc) as tc:
        with (
            tc.tile_pool(name="const", bufs=1) as cpool,
            tc.tile_pool(name="xpool", bufs=1) as xpool,
            tc.tile_pool(name="upool", bufs=3) as upool,
            tc.tile_pool(name="boxp", bufs=3) as boxp,
            tc.tile_pool(name="tfp", bufs=4) as tfp,
            tc.tile_pool(name="spool", bufs=1) as spool,
            tc.tile_pool(name="opool", bufs=8) as opool,
            tc.tile_pool(name="psr", bufs=1, space="PSUM") as psr,
            tc.tile_pool(name="psc", bufs=6, space="PSUM") as psc,
            tc.tile_pool(name="dram", bufs=1, space="DRAM") as dram,
        ):
            # ---- dummy collective, triggered first: absorbs the NRT entry
            # barrier + first-collective ncfw setup (~25-50us) under compute,
            # so the real stats AllGather later starts in ~1us ----
            dcin = dram.tile([128, 2], F32, name="dcin")
            dcout = dram.tile([128 * 4, 2], F32, name="dcout")
            nc.gpsimd.collective_compute(
                "AllGather",
                mybir.AluOpType.bypass,
                replica_groups=CC_GROUPS,
                ins=[dcin[:].opt()],
                outs=[dcout[:].opt()],
            )

            # ---- constants: criticality-ordered. Early consumers (warmup,
            # r4, rc) load on the sync queue ahead of the images; bulky /
            # late-consumed ones go on the gpsimd queue ----
            xall = xpool.tile([128, NIMG_L * NPIX], F16, name="xall")

            def ximg(m):
                nc.sync.dma_start(
                    xall[:, m * NPIX : (m + 1) * NPIX],
                    xh[:, m * NPIX : (m + 1) * NPIX],
                )

            ximg(0)
            ximg(1)
            ones128 = cpool.tile([128, 128], F16, name="ones128")
            nc.sync.dma_start(ones128[:], ones128d[:])
            compt = cpool.tile([128, NPIX], F16, name="compt")
            nc.sync.dma_start(compt[:], comp16d[:])
            idt = cpool.tile([128, 128], F16, name="idt")
            nc.sync.dma_start(idt[:], id128d[:])
            for _m in range(2, NIMG_L):
                ximg(_m)
            c32 = cpool.tile([128, 3], F32, name="c32")
            nc.sync.dma_start(c32[:], cst32d[:])
            wtile = cpool.tile([128, 9 * 128], F16, name="wtile")
            nc.gpsimd.dma_start(wtile[:], wt[:])
            onert = cpool.tile([1, 128], F16, name="onert")
            nc.gpsimd.dma_start(onert[:], onesrd[:])
            cmapt = cpool.tile([1, 840], F16, name="cmapt")
            nc.gpsimd.dma_start(cmapt[:], cmap16d[:])
            cm3 = cmapt[:].rearrange("p (a c) -> p a c", c=HP)

            x3 = xall[:].rearrange("p (n a b) -> p n a b", a=HP, b=HP)

            s_sb = spool.tile([128, NIMG_L * NV], F32, name="s_sb")
            sums16 = spool.tile([128, 2 * NIMG_L], F32, name="sums16")
            sumsq = spool.tile([128, NIMG_L], F32, name="sumsq")

            # ---- PE warmup: flip HAM to 8/8 during the input-DMA window.
            # Stationary operand comes from a memset tile so the warmup has
            # no DMA dependency at all ----
            wsrc = cpool.tile([128, 128], F16, name="wsrc")
            nc.vector.memset(wsrc[:], 0.25)
            warm = psr.tile([128, 1024], F32, name="warm", tag="r4")
            for i in range(N_WARM):
                nc.tensor.matmul(
                    warm[:, 0:128], wsrc[:], wsrc[:], start=True, stop=True,
                    skip_group_check=True,
                )

            # ---- ACT spline-table preload (first activation pays ~1.3us);
            # memset feeds it so it has no input-DMA dependency ----
            tscr = spool.tile([128, 8], F32, name="tscr")
            nc.vector.memset(tscr[:, 0:4], 1.0)
            nc.scalar.activation(
                tscr[:, 4:8], tscr[:, 0:4], mybir.ActivationFunctionType.Square
            )

            # ---- x^2: first two images on DVE (fast startup), rest on ACT;
            # emitted up-front so the ACT FIFO serves them before the drains ----
            uts = []
            for m in range(NIMG_L):
                ut = upool.tile([128, NPIX], F16, name=f"u{m}", tag="u")
                xs = xall[:, m * NPIX : (m + 1) * NPIX]
                if m < 1:
                    nc.vector.tensor_mul(ut[:], xs, xs)
                else:
                    nc.scalar.activation(
                        ut[:], xs, mybir.ActivationFunctionType.Square
                    )
                uts.append(ut)

            tfs = [None] * NIMG_L

            def box_chain(m):
                """r4 matmul + centered cast + separable 3x3 box filter for
                image m; leaves tf (t1 - 128*count on the 30-grid) in tfs[m].
                split=True runs it in two y-halves so the first conv chunk's
                injects unblock ~2x sooner."""
                r4 = psr.tile([128, 1024], F32, name=f"r4_{m}", tag="r4")
                for lo, hi in ((0, 512), (512, NPIX)):
                    nc.tensor.matmul(
                        r4[:, lo:hi],
                        ones128[:],
                        uts[m][:, lo:hi],
                        start=True,
                        stop=True,
                        skip_group_check=True,
                    )
                rc = boxp.tile([128, NPIX], F16, name=f"rc{m}", tag="rc")
                vv = boxp.tile([128, 840], F16, name=f"vv{m}", tag="vv")
                te = boxp.tile([128, 840], F16, name=f"te{m}", tag="te")
                tf = tfp.tile([128, 840], F16, name=f"tf{m}", tag="tf")
                split = False
                halves = ((0, NPIX),)
                for hi_, (ra, rb) in enumerate(halves):
                    nc.vector.tensor_sub(
                        rc[:, ra:rb], r4[:, ra:rb], compt[:, ra:rb]
                    )
                    va, vb = (0, 420) if (split and hi_ == 0) else (
                        (420, 840) if split else (0, 840)
                    )
                    nc.vector.tensor_add(
                        vv[:, va:vb], rc[:, va:vb], rc[:, va + 30 : vb + 30]
                    )
                    nc.vector.tensor_add(
                        vv[:, va:vb], vv[:, va:vb], rc[:, va + 60 : vb + 60]
                    )
                    tb = min(vb, 838)
                    if split and hi_ == 0:
                        tb = 418  # row 13 max valid col; avoids touching half 1
                    nc.vector.tensor_add(
                        te[:, va:tb], vv[:, va:tb], vv[:, va + 2 : tb + 2]
                    )
                    if split:
                        nc.gpsimd.tensor_add(
                            tf[:, va:tb], te[:, va:tb], vv[:, va + 1 : tb + 1]
                        )
                    else:
                        nc.gpsimd.tensor_add(
                            tf[:, 0:420], te[:, 0:420], vv[:, 1:421]
                        )
                        nc.vector.tensor_add(
                            tf[:, 420:838], te[:, 420:838], vv[:, 421:839]
                        )
                tfs[m] = tf

            def conv_chunk(b):
                """Conv accumulation groups for images 2b, 2b+1. One psum
                BANK per (img, yt-half). No t1 injection here: t1 (tf) is
                added during the drain, so conv never waits on the box
                chains. Returns the deferred drain emitter."""
                ms = (2 * b, 2 * b + 1)
                pss = {}
                for m in ms:
                    for yt in range(2):
                        pss[(m, yt)] = psc.tile(
                            [128, 512], F32, name=f"ps{m}_{yt}", tag="ps"
                        )
                # conv: k-major so each weight load serves 4 matmuls
                for k in range(9):
                    dy, dx = divmod(k, 3)
                    for m in ms:
                        for yt in range(2):
                            y0 = yt * 14
                            nc.tensor.matmul(
                                pss[(m, yt)][:, 0:392],
                                wtile[:, k * 128 : (k + 1) * 128],
                                x3[:, m, y0 + dy : y0 + dy + 14, dx : dx + 28],
                                start=(k == 0),
                                stop=False,
                                skip_group_check=True,
                            )
                # countmap (uncenter) closes the groups
                for m in ms:
                    for yt in range(2):
                        nc.tensor.matmul(
                            pss[(m, yt)][:, 0:392],
                            onert[:],
                            cm3[:, 14 * yt : 14 * yt + 14, 0:28],
                            start=False,
                            stop=True,
                            skip_group_check=True,
                        )

                def drains():
                    # DVE: s = tf + psum (adds t1, casts, accum -> sums)
                    for m in ms:
                        t13v = tfs[m]
                        for yt in range(2):
                            off = m * NV + yt * 392
                            sdst = AP(
                                s_sb.tensor,
                                s_sb.offset + off,
                                [[NIMG_L * NV, 128], [28, 14], [1, 28]],
                            )
                            tfv = AP(
                                t13v.tensor,
                                t13v.offset + yt * 420,
                                [[840, 128], [30, 14], [1, 28]],
                            )
                            psv = AP(
                                pss[(m, yt)].tensor,
                                pss[(m, yt)].offset,
                                [[512, 128], [28, 14], [1, 28]],
                            )
                            nc.vector.scalar_tensor_tensor(
                                sdst,
                                tfv,
                                1.0,
                                psv,
                                op0=mybir.AluOpType.mult,
                                op1=mybir.AluOpType.add,
                                accum_out=sums16[:, 2 * m + yt : 2 * m + yt + 1],
                            )
                    for m in ms:
                        blk = m * NV
                        sq_scr = opool.tile([128, NV], F32, name=f"sq{m}", tag="sq")
                        nc.scalar.activation(
                            sq_scr[:],
                            s_sb[:, blk : blk + NV],
                            mybir.ActivationFunctionType.Square,
                            accum_out=sumsq[:, m : m + 1],
                        )

                return drains

            box_chain(0)
            d0 = conv_chunk(0)
            box_chain(1)
            box_chain(2)
            box_chain(3)
            d0()
            d1 = conv_chunk(1)
            box_chain(4)
            box_chain(5)
            d1()
            d2 = conv_chunk(2)
            box_chain(6)
            box_chain(7)
            d2()
            d3 = conv_chunk(3)
            d3()

            # ---- stats: local fold -> 4-rank AllGather -> global fold ----
            st2 = spool.tile([128, 2], F32, name="st2")
            nc.vector.tensor_reduce(
                out=st2[:, 0:1], in_=sums16[:], op=mybir.AluOpType.add,
                axis=mybir.AxisListType.X,
            )
            nc.vector.tensor_reduce(
                out=st2[:, 1:2], in_=sumsq[:], op=mybir.AluOpType.add,
                axis=mybir.AxisListType.X,
            )
            cin = dram.tile([128, 2], F32, name="cin")
            cout = dram.tile([128 * 4, 2], F32, name="cout")
            nc.sync.dma_start(cin[:], st2[:])
            nc.gpsimd.collective_compute(
                "AllGather",
                mybir.AluOpType.bypass,
                replica_groups=CC_GROUPS,
                ins=[cin[:].opt()],
                outs=[cout[:].opt()],
            )
            g = spool.tile([128, 8], F32, name="g")
            nc.sync.dma_start(
                g[:], AP(cout.tensor, cout.offset, [[2, 128], [256, 4], [1, 2]])
            )
            gs = spool.tile([128, 2], F32, name="gs")
            nc.vector.tensor_add(gs[:], g[:, 0:2], g[:, 2:4])
            nc.vector.tensor_add(gs[:], gs[:], g[:, 4:6])
            nc.vector.tensor_add(gs[:], gs[:], g[:, 6:8])

            ab = spool.tile([128, 8], F32, name="ab")
            mean = ab[:, 0:1]
            qn = ab[:, 1:2]
            nc.vector.tensor_scalar_mul(mean, gs[:, 0:1], 1.0 / NHW)
            nc.vector.tensor_scalar_mul(qn, gs[:, 1:2], 1.0 / NHW)
            var = ab[:, 2:3]
            nc.vector.scalar_tensor_tensor(
                var, mean, 1.0, mean, op0=mybir.AluOpType.mult,
                op1=mybir.AluOpType.mult,
            )
            nc.vector.tensor_sub(var, qn, var)
            sd = ab[:, 3:4]
            nc.scalar.activation(
                sd, var, mybir.ActivationFunctionType.Sqrt, bias=c32[:, 2:3]
            )
            abv = spool.tile([128, 2], F32, name="abv")
            A = abv[:, 0:1]
            B = abv[:, 1:2]
            nc.vector.reciprocal(A, sd)
            nc.vector.tensor_mul(A, A, c32[:, 0:1])
            nc.vector.scalar_tensor_tensor(
                B, mean, 1.0, A, op0=mybir.AluOpType.mult, op1=mybir.AluOpType.mult
            )
            nc.vector.tensor_sub(B, c32[:, 1:2], B)

            # ---- normalize + store (engine rotation) ----
            for m in range(NIMG_L):
                blk = m * NV
                o = opool.tile([128, NV], F16, name=f"o{m}", tag="o")
                if m % 2 == 0:
                    nc.vector.tensor_scalar(
                        o[:],
                        s_sb[:, blk : blk + NV],
                        A,
                        B,
                        op0=mybir.AluOpType.mult,
                        op1=mybir.AluOpType.add,
                    )
                else:
                    nc.scalar.activation(
                        o[:],
                        s_sb[:, blk : blk + NV],
                        mybir.ActivationFunctionType.Identity,
                        bias=B,
                        scale=A,
                    )
                dst = AP(y.ap().tensor, m * 128 * NV, [[NV, 128], [1, NV]])
                eng = nc.sync if m % 2 == 0 else nc.scalar
                eng.dma_start(dst, o[:])
    return nc


def _prep_inputs(x, w, gamma, beta):
    x = np.asarray(x, np.float32)
    w = np.asarray(w, np.float32)
    gamma = np.asarray(gamma, np.float32)
    beta = np.asarray(beta, np.float32)

    xp = np.zeros((32, 128, HP, HP), np.float32)
    xp[:, :, 1:29, 1:29] = x

    ones128 = np.ones((128, 128), np.float16)
    id128 = np.eye(128, dtype=np.float16)
    onesr = np.ones((1, 128), np.float16)

    pidx = np.arange(NPIX)
    py, px = pidx // HP, pidx % HP
    valid = (py >= 1) & (py <= 28) & (px >= 1) & (px <= 28)
    comp16 = np.broadcast_to((128.0 * valid).astype(np.float16), (128, NPIX)).copy()

    jj = np.arange(840)
    jy, jx = jj // HP, jj % HP
    cy = np.where((jy == 0) | (jy == 27), 2, 3)
    cx = np.where((jx == 0) | (jx == 27), 2, 3)
    used = (jy < 28) & (jx < 28)
    cmap16 = np.where(used, 128.0 * cy * cx, 0.0).astype(np.float16)[None, :]

    maps = []
    for core in range(N_CORES):
        cg, bg = core // 4, core % 4
        xs = xp[bg * NIMG_L : (bg + 1) * NIMG_L]
        xhc = np.ascontiguousarray(xs.transpose(1, 0, 2, 3)).reshape(
            128, NIMG_L * NPIX
        )
        wc = (2.0 * w[cg * 128 : (cg + 1) * 128]).reshape(128, 128, 9)
        wtc = np.ascontiguousarray(wc.transpose(1, 2, 0)).reshape(128, 9 * 128)
        cst32 = np.zeros((128, 3), np.float32)
        cst32[:, 0] = gamma[cg * 128 : (cg + 1) * 128]
        cst32[:, 1] = beta[cg * 128 : (cg + 1) * 128]
        cst32[:, 2] = EPS
        maps.append(
            {
                "xh": xhc.astype(np.float16),
                "wt": wtc.astype(np.float16),
                "ones128": ones128,
                "id128": id128,
                "onesr": onesr,
                "comp16": comp16,
                "cmap16": cmap16,
                "cst32": cst32,
            }
        )
    return maps


_NC_CACHE = []


def _assemble(results):
    out = np.empty((32, 256, 28, 28), np.float32)
    for core in range(N_CORES):
        cg, bg = core // 4, core % 4
        out[bg * NIMG_L : (bg + 1) * NIMG_L, cg * 128 : (cg + 1) * 128] = (
            results[core]["y"].astype(np.float32)
        )
    return out


def kernel(x, w, gamma, beta):
    if not _NC_CACHE:
        _NC_CACHE.append(_build_nc())
    nc = _NC_CACHE[0]
    maps = _prep_inputs(x, w, gamma, beta)
    res = run_bass_kernel_spmd(nc, maps, core_ids=list(range(N_CORES)))
    return _assemble(res.results)


# revision 15
# speedup vs baseline: 1.1273x; 1.1273x over previous
"""EuclidConv + training-mode BatchNorm on 8 Trainium2 NeuronCores.

Math (reference): out = BN(2*conv(x,w) + conv(x^2, ones3x3) + ||w_f||^2),
BN over global-batch stats. The per-filter ||w||^2 term is channel-constant,
so BN's mean subtraction cancels it exactly -> never computed.

Sharding: HYBRID. core c -> (chgrp = c//4, bgrp = c%4): 128 of 256 output
channels x 8 of 32 images. This gives full-width M=128 matmuls (the pure
channel-sharded layout only fills 32 of 128 PE output columns), 4x less PE
streaming. The price: BN statistics must be reduced across the 4 bgrps that
share a channel group -> one tiny 4-rank AllGather of [128,2] partial
(sum, sumsq) + local fold.

Per image m (padded 30x30 grid, fp16):
  u_m = x_m^2                                      (img0 DVE, rest ACT)
  r4 psum = ones128.T @ u_m    (channel sums of x^2, replicated over all
                                128 partitions; 2 MMs)
  rc = r4 - 128*validmap       (DVE, fp16, centered for precision)
  box filter: vv = 3-tap vertical (DVE, stride-30 = pair-aligned 2x mode),
  te = vv[0]+vv[+2] (2x), tf = te + vv[+1] (GpSimd - odd offset would be
  1x-mode on DVE anyway, and DVE is the busier engine)
Conv accumulation group per (img, yt-half) [128,392] psum:
  identity.T @ tf_view         (start=True: seeds psum with t1 - 128*count)
  sum_k (2w)_k.T @ x_view      (9 offsets, full M=128)
  ones1.T @ cmap_view          (stop=True: re-adds 128*count)
Drain: ACT copy psum->s_sb with accum S; ACT square with accum Q.
Stats: fold S,Q over 8 local images -> [128,2]; 4-rank AllGather via HBM
bounce; fold 4 ranks; A = gamma*rsqrt(var+eps), B = beta - mean*A;
normalize out = s*A+B (DVE/ACT alternation, fp16) -> dual-queue DMA out.

Collective latency: the first collective of an execution pays a ~25-45us
NRT/ncfw entry barrier that runs on TOPSP concurrently with compute -- a
dummy AllGather triggered at kernel start absorbs it, so the real stats
AllGather starts ~1-2us after its trigger. Chunk-0 box chains are split
into y-halves and its conv is img-major so the PE pipeline fills ~8us
sooner; a memset-fed matmul warmup flips the PE HAM clock gate to 2.4GHz
during the input-DMA window.

Host-side prep is layout/sharding only: pad+transpose+cast of x, weight
transpose/scale, constant masks.
"""
import json

import numpy as np

import subprocess as _subprocess

import concourse.bass as bass
import concourse.bass_utils as _bass_utils
import concourse.mybir as mybir
import concourse.tile as tile
from concourse.ap import AP
from concourse.bass_utils import run_bass_kernel_spmd
from concourse.vector_clock import ScopedClock, VectorClock

F16 = mybir.dt.float16
F32 = mybir.dt.float32


class _WalrusLdwOpt:
    """Enable walrus's LDWEIGHTS dedup pass for this kernel's compiles.
    bass_utils hardcodes --enable-ldw-opt=false; this kernel issues runs of
    4+ matmuls sharing one stationary operand, where redundant per-matmul
    weight reloads serialize ~110ns each on the PE."""

    def __getattr__(self, name):
        return getattr(_subprocess, name)

    def check_call(self, argv, **kw):
        if (
            isinstance(argv, list)
            and argv
            and "walrus_driver" in str(argv[0])
        ):
            argv = [
                "--enable-ldw-opt=true" if a == "--enable-ldw-opt=false" else a
                for a in argv
            ]
        return _subprocess.check_call(argv, **kw)


_bass_utils.subprocess = _WalrusLdwOpt()

N_CORES = 8
NIMG_L = 8  # images per core
HP = 30
NPIX = HP * HP
NV = 28 * 28
NHW = 32 * NV  # global batch pixels per channel
EPS = 1e-5
CC_GROUPS = [[0, 1, 2, 3], [4, 5, 6, 7]]
N_WARM = 30

_split_ctr = [0]


def _split_waits_json(bir: bytes, max_waits: int = 1) -> bytes:
    """This container's walrus rejects instructions with >1 sync wait.
    Hoist excess waits onto EventSemaphore instructions inserted before the
    offender on the same engine stream."""
    m = json.loads(bir)
    for f in m["functions"]:
        for bb in f["blocks"]:
            newinsts = []
            for ins in bb["instructions"]:
                si = ins.get("sync_info")
                if si:
                    waits = si.get("on_wait") or []
                    if len(waits) > max_waits:
                        extra, keep = waits[:-max_waits], waits[-max_waits:]
                        for w_ in extra:
                            _split_ctr[0] += 1
                            newinsts.append(
                                {
                                    "debug": ins.get("debug", 0),
                                    "engine": ins["engine"],
                                    "ins": [],
                                    "outs": [],
                                    "name": f"antsplitw-{_split_ctr[0]}",
                                    "opcode": "EventSemaphore",
                                    "sync_info": {"on_update": [], "on_wait": [w_]},
                                }
                            )
                        si["on_wait"] = keep
                newinsts.append(ins)
            bb["instructions"] = newinsts
    return json.dumps(m).encode()


class _PatchedBass(bass.Bass):
    def to_json_bytes(self):
        return _split_waits_json(super().to_json_bytes())


class _SplitDrainTileContext(tile.TileContext):
    """Split the tile-exit drain's waits into single-wait drains (same
    walrus limitation as above)."""

    def _drain_and_barrier(self, tick_clock, wait_clock):
        g = tick_clock.global_clock
        n = len(g)
        for i in range(n):
            if g[i] > 0:
                vec = [0] * n
                vec[i] = g[i]
                d = self.nc.sync.drain()
                wait_clock.add_sem_waits(d.ins, ScopedClock({None: VectorClock(vec)}))
        self.nc.sync.drain()
        self.nc.all_engine_barrier()
        assert self.sems is not None
        popped = self.nc._tile_sem_poison_stack.pop()
        assert popped is self._sem_poison
        self.nc.clear_and_free_semaphores(list(self.sems.allocated().values()))
        self.nc.all_engine_barrier()


def _build_nc():
    nc = _PatchedBass(num_devices=N_CORES)
    xh = nc.dram_tensor("xh", [128, NIMG_L * NPIX], F16, kind="ExternalInput")
    wt = nc.dram_tensor("wt", [128, 9 * 128], F16, kind="ExternalInput")
    ones128d = nc.dram_tensor("ones128", [128, 128], F16, kind="ExternalInput")
    id128d = nc.dram_tensor("id128", [128, 128], F16, kind="ExternalInput")
    onesrd = nc.dram_tensor("onesr", [1, 128], F16, kind="ExternalInput")
    comp16d = nc.dram_tensor("comp16", [128, NPIX], F16, kind="ExternalInput")
    cmap16d = nc.dram_tensor("cmap16", [1, 840], F16, kind="ExternalInput")
    cst32d = nc.dram_tensor("cst32", [128, 3], F32, kind="ExternalInput")
    y = nc.dram_tensor("y", [NIMG_L, 128, 28, 28], F16, kind="ExternalOutput")

    with _SplitDrainTileContext(nc) as tc:
        with (
            tc.tile_pool(name="const", bufs=1) as cpool,
            tc.tile_pool(name="xpool", bufs=1) as xpool,
            tc.tile_pool(name="upool", bufs=3) as upool,
            tc.tile_pool(name="boxp", bufs=3) as boxp,
            tc.tile_pool(name="tfp", bufs=4) as tfp,
            tc.tile_pool(name="spool", bufs=1) as spool,
            tc.tile_pool(name="opool", bufs=8) as opool,
            tc.tile_pool(name="psr", bufs=2, space="PSUM") as psr,
            tc.tile_pool(name="psc", bufs=4, space="PSUM") as psc,
            tc.tile_pool(name="dram", bufs=1, space="DRAM") as dram,
        ):
            # ---- dummy collective, triggered first: absorbs the NRT entry
            # barrier + first-collective ncfw setup (~25-50us) under compute,
            # so the real stats AllGather later starts in ~1us ----
            dcin = dram.tile([128, 2], F32, name="dcin")
            dcout = dram.tile([128 * 4, 2], F32, name="dcout")
            nc.gpsimd.collective_compute(
                "AllGather",
                mybir.AluOpType.bypass,
                replica_groups=CC_GROUPS,
                ins=[dcin[:].opt()],
                outs=[dcout[:].opt()],
            )

            # ---- constants: criticality-ordered. Early consumers (warmup,
            # r4, rc) load on the sync queue ahead of the images; bulky /
            # late-consumed ones go on the gpsimd queue ----
            xall = xpool.tile([128, NIMG_L * NPIX], F16, name="xall")

            def ximg(m):
                nc.sync.dma_start(
                    xall[:, m * NPIX : (m + 1) * NPIX],
                    xh[:, m * NPIX : (m + 1) * NPIX],
                )

            ximg(0)
            ximg(1)
            ones128 = cpool.tile([128, 128], F16, name="ones128")
            nc.sync.dma_start(ones128[:], ones128d[:])
            compt = cpool.tile([128, NPIX], F16, name="compt")
            nc.sync.dma_start(compt[:], comp16d[:])
            idt = cpool.tile([128, 128], F16, name="idt")
            nc.sync.dma_start(idt[:], id128d[:])
            for _m in range(2, NIMG_L):
                ximg(_m)
            c32 = cpool.tile([128, 3], F32, name="c32")
            nc.sync.dma_start(c32[:], cst32d[:])
            wtile = cpool.tile([128, 9 * 128], F16, name="wtile")
            nc.gpsimd.dma_start(wtile[:], wt[:])
            onert = cpool.tile([1, 128], F16, name="onert")
            nc.gpsimd.dma_start(onert[:], onesrd[:])
            cmapt = cpool.tile([1, 840], F16, name="cmapt")
            nc.gpsimd.dma_start(cmapt[:], cmap16d[:])
            cm3 = cmapt[:].rearrange("p (a c) -> p a c", c=HP)

            x3 = xall[:].rearrange("p (n a b) -> p n a b", a=HP, b=HP)

            s_sb = spool.tile([128, NIMG_L * NV], F32, name="s_sb")
            sums16 = spool.tile([128, 2 * NIMG_L], F32, name="sums16")
            sumsq = spool.tile([128, NIMG_L], F32, name="sumsq")

            # ---- PE warmup: flip HAM to 8/8 during the input-DMA window.
            # Stationary operand comes from a memset tile so the warmup has
            # no DMA dependency at all ----
            wsrc = cpool.tile([128, 128], F16, name="wsrc")
            nc.vector.memset(wsrc[:], 0.25)
            warm = psr.tile([128, 1024], F32, name="warm", tag="r4")
            for i in range(N_WARM):
                nc.tensor.matmul(
                    warm[:, 0:128], wsrc[:], wsrc[:], start=True, stop=True,
                    skip_group_check=True,
                )

            # ---- ACT spline-table preload (first activation pays ~1.3us);
            # memset feeds it so it has no input-DMA dependency ----
            tscr = spool.tile([128, 8], F32, name="tscr")
            nc.vector.memset(tscr[:, 0:4], 1.0)
            nc.scalar.activation(
                tscr[:, 4:8], tscr[:, 0:4], mybir.ActivationFunctionType.Square
            )

            # ---- x^2: first two images on DVE (fast startup), rest on ACT;
            # emitted up-front so the ACT FIFO serves them before the drains ----
            uts = []
            for m in range(NIMG_L):
                ut = upool.tile([128, NPIX], F16, name=f"u{m}", tag="u")
                xs = xall[:, m * NPIX : (m + 1) * NPIX]
                if m < 1:
                    nc.vector.tensor_mul(ut[:], xs, xs)
                else:
                    nc.scalar.activation(
                        ut[:], xs, mybir.ActivationFunctionType.Square
                    )
                uts.append(ut)

            tfs = [None] * NIMG_L

            def box_chain(m):
                """r4 matmul + centered cast + separable 3x3 box filter for
                image m; leaves tf (t1 - 128*count on the 30-grid) in tfs[m].
                split=True runs it in two y-halves so the first conv chunk's
                injects unblock ~2x sooner."""
                r4 = psr.tile([128, 1024], F32, name=f"r4_{m}", tag="r4")
                for lo, hi in ((0, 512), (512, NPIX)):
                    nc.tensor.matmul(
                        r4[:, lo:hi],
                        ones128[:],
                        uts[m][:, lo:hi],
                        start=True,
                        stop=True,
                        skip_group_check=True,
                    )
                rc = boxp.tile([128, NPIX], F16, name=f"rc{m}", tag="rc")
                vv = boxp.tile([128, 840], F16, name=f"vv{m}", tag="vv")
                te = boxp.tile([128, 840], F16, name=f"te{m}", tag="te")
                tf = tfp.tile([128, 840], F16, name=f"tf{m}", tag="tf")
                split = False
                halves = ((0, NPIX),)
                for hi_, (ra, rb) in enumerate(halves):
                    nc.vector.tensor_sub(
                        rc[:, ra:rb], r4[:, ra:rb], compt[:, ra:rb]
                    )
                    va, vb = (0, 420) if (split and hi_ == 0) else (
                        (420, 840) if split else (0, 840)
                    )
                    nc.vector.tensor_add(
                        vv[:, va:vb], rc[:, va:vb], rc[:, va + 30 : vb + 30]
                    )
                    nc.vector.tensor_add(
                        vv[:, va:vb], vv[:, va:vb], rc[:, va + 60 : vb + 60]
                    )
                    tb = min(vb, 838)
                    if split and hi_ == 0:
                        tb = 418  # row 13 max valid col; avoids touching half 1
                    nc.vector.tensor_add(
                        te[:, va:tb], vv[:, va:tb], vv[:, va + 2 : tb + 2]
                    )
                    nc.gpsimd.tensor_add(
                        tf[:, 0:838], te[:, 0:838], vv[:, 1:839]
                    )
                tfs[m] = tf

            def conv_chunk(b):
                """Conv accumulation groups for images 2b, 2b+1. One psum
                BANK per (img, yt-half). No t1 injection here: t1 (tf) is
                added during the drain, so conv never waits on the box
                chains. Returns the deferred drain emitter."""
                ms = (2 * b, 2 * b + 1)
                pss = {}
                for m in ms:
                    for yt in range(2):
                        pss[(m, yt)] = psc.tile(
                            [128, 512], F32, name=f"ps{m}_{yt}", tag="ps"
                        )
                # conv: k-major so each weight load serves 4 matmuls
                for k in range(9):
                    dy, dx = divmod(k, 3)
                    for m in ms:
                        for yt in range(2):
                            y0 = yt * 14
                            nc.tensor.matmul(
                                pss[(m, yt)][:, 0:392],
                                wtile[:, k * 128 : (k + 1) * 128],
                                x3[:, m, y0 + dy : y0 + dy + 14, dx : dx + 28],
                                start=(k == 0),
                                stop=False,
                                skip_group_check=True,
                            )
                # countmap (uncenter) closes the groups
                for m in ms:
                    for yt in range(2):
                        nc.tensor.matmul(
                            pss[(m, yt)][:, 0:392],
                            onert[:],
                            cm3[:, 14 * yt : 14 * yt + 14, 0:28],
                            start=False,
                            stop=True,
                            skip_group_check=True,
                        )

                def drains():
                    # DVE: s = tf + psum (adds t1, casts, accum -> sums)
                    for m in ms:
                        t13v = tfs[m]
                        for yt in range(2):
                            off = m * NV + yt * 392
                            sdst = AP(
                                s_sb.tensor,
                                s_sb.offset + off,
                                [[NIMG_L * NV, 128], [28, 14], [1, 28]],
                            )
                            tfv = AP(
                                t13v.tensor,
                                t13v.offset + yt * 420,
                                [[840, 128], [30, 14], [1, 28]],
                            )
                            psv = AP(
                                pss[(m, yt)].tensor,
                                pss[(m, yt)].offset,
                                [[512, 128], [28, 14], [1, 28]],
                            )
                            nc.vector.scalar_tensor_tensor(
                                sdst,
                                tfv,
                                1.0,
                                psv,
                                op0=mybir.AluOpType.mult,
                                op1=mybir.AluOpType.add,
                                accum_out=sums16[:, 2 * m + yt : 2 * m + yt + 1],
                            )
                    for m in ms:
                        blk = m * NV
                        sq_scr = opool.tile([128, NV], F32, name=f"sq{m}", tag="sq")
                        nc.scalar.activation(
                            sq_scr[:],
                            s_sb[:, blk : blk + NV],
                            mybir.ActivationFunctionType.Square,
                            accum_out=sumsq[:, m : m + 1],
                        )

                return drains

            box_chain(0)
            box_chain(1)
            d0 = conv_chunk(0)
            d0()
            box_chain(2)
            box_chain(3)
            d1 = conv_chunk(1)
            d1()
            box_chain(4)
            box_chain(5)
            d2 = conv_chunk(2)
            d2()
            box_chain(6)
            box_chain(7)
            d3 = conv_chunk(3)
            d3()

            # ---- stats: local fold -> 4-rank AllGather -> global fold ----
            st2 = spool.tile([128, 2], F32, name="st2")
            nc.vector.tensor_reduce(
                out=st2[:, 0:1], in_=sums16[:], op=mybir.AluOpType.add,
                axis=mybir.AxisListType.X,
            )
            nc.vector.tensor_reduce(
                out=st2[:, 1:2], in_=sumsq[:], op=mybir.AluOpType.add,
                axis=mybir.AxisListType.X,
            )
            cin = dram.tile([128, 2], F32, name="cin")
            cout = dram.tile([128 * 4, 2], F32, name="cout")
            nc.sync.dma_start(cin[:], st2[:])
            nc.gpsimd.collective_compute(
                "AllGather",
                mybir.AluOpType.bypass,
                replica_groups=CC_GROUPS,
                ins=[cin[:].opt()],
                outs=[cout[:].opt()],
            )
            g = spool.tile([128, 8], F32, name="g")
            nc.sync.dma_start(
                g[:], AP(cout.tensor, cout.offset, [[2, 128], [256, 4], [1, 2]])
            )
            gs = spool.tile([128, 2], F32, name="gs")
            nc.vector.tensor_add(gs[:], g[:, 0:2], g[:, 2:4])
            nc.vector.tensor_add(gs[:], gs[:], g[:, 4:6])
            nc.vector.tensor_add(gs[:], gs[:], g[:, 6:8])

            ab = spool.tile([128, 8], F32, name="ab")
            mean = ab[:, 0:1]
            qn = ab[:, 1:2]
            nc.vector.tensor_scalar_mul(mean, gs[:, 0:1], 1.0 / NHW)
            nc.vector.tensor_scalar_mul(qn, gs[:, 1:2], 1.0 / NHW)
            var = ab[:, 2:3]
            nc.vector.scalar_tensor_tensor(
                var, mean, 1.0, mean, op0=mybir.AluOpType.mult,
                op1=mybir.AluOpType.mult,
            )
            nc.vector.tensor_sub(var, qn, var)
            sd = ab[:, 3:4]
            nc.scalar.activation(
                sd, var, mybir.ActivationFunctionType.Sqrt, bias=c32[:, 2:3]
            )
            abv = spool.tile([128, 2], F32, name="abv")
            A = abv[:, 0:1]
            B = abv[:, 1:2]
            nc.vector.reciprocal(A, sd)
            nc.vector.tensor_mul(A, A, c32[:, 0:1])
            nc.vector.scalar_tensor_tensor(
                B, mean, 1.0, A, op0=mybir.AluOpType.mult, op1=mybir.AluOpType.mult
            )
            nc.vector.tensor_sub(B, c32[:, 1:2], B)

            # ---- normalize + store (engine rotation) ----
            for m in range(NIMG_L):
                blk = m * NV
                o = opool.tile([128, NV], F16, name=f"o{m}", tag="o")
                if m % 2 == 0:
                    nc.vector.tensor_scalar(
                        o[:],
                        s_sb[:, blk : blk + NV],
                        A,
                        B,
                        op0=mybir.AluOpType.mult,
                        op1=mybir.AluOpType.add,
                    )
                else:
                    nc.scalar.activation(
                        o[:],
                        s_sb[:, blk : blk + NV],
                        mybir.ActivationFunctionType.Identity,
                        bias=B,
                        scale=A,
                    )
                dst = AP(y.ap().tensor, m * 128 * NV, [[NV, 128], [1, NV]])
                eng = nc.sync if m % 2 == 0 else nc.scalar
                eng.dma_start(dst, o[:])
    return nc


def _prep_inputs(x, w, gamma, beta):
    x = np.asarray(x, np.float32)
    w = np.asarray(w, np.float32)
    gamma = np.asarray(gamma, np.float32)
    beta = np.asarray(beta, np.float32)

    xp = np.zeros((32, 128, HP, HP), np.float32)
    xp[:, :, 1:29, 1:29] = x

    ones128 = np.ones((128, 128), np.float16)
    id128 = np.eye(128, dtype=np.float16)
    onesr = np.ones((1, 128), np.float16)

    pidx = np.arange(NPIX)
    py, px = pidx // HP, pidx % HP
    valid = (py >= 1) & (py <= 28) & (px >= 1) & (px <= 28)
    comp16 = np.broadcast_to((128.0 * valid).astype(np.float16), (128, NPIX)).copy()

    jj = np.arange(840)
    jy, jx = jj // HP, jj % HP
    cy = np.where((jy == 0) | (jy == 27), 2, 3)
    cx = np.where((jx == 0) | (jx == 27), 2, 3)
    used = (jy < 28) & (jx < 28)
    cmap16 = np.where(used, 128.0 * cy * cx, 0.0).astype(np.float16)[None, :]

    maps = []
    for core in range(N_CORES):
        cg, bg = core // 4, core % 4
        xs = xp[bg * NIMG_L : (bg + 1) * NIMG_L]
        xhc = np.ascontiguousarray(xs.transpose(1, 0, 2, 3)).reshape(
            128, NIMG_L * NPIX
        )
        wc = (2.0 * w[cg * 128 : (cg + 1) * 128]).reshape(128, 128, 9)
        wtc = np.ascontiguousarray(wc.transpose(1, 2, 0)).reshape(128, 9 * 128)
        cst32 = np.zeros((128, 3), np.float32)
        cst32[:, 0] = gamma[cg * 128 : (cg + 1) * 128]
        cst32[:, 1] = beta[cg * 128 : (cg + 1) * 128]
        cst32[:, 2] = EPS
        maps.append(
            {
                "xh": xhc.astype(np.float16),
                "wt": wtc.astype(np.float16),
                "ones128": ones128,
                "id128": id128,
                "onesr": onesr,
                "comp16": comp16,
                "cmap16": cmap16,
                "cst32": cst32,
            }
        )
    return maps


_NC_CACHE = []


def _assemble(results):
    out = np.empty((32, 256, 28, 28), np.float32)
    for core in range(N_CORES):
        cg, bg = core // 4, core % 4
        out[bg * NIMG_L : (bg + 1) * NIMG_L, cg * 128 : (cg + 1) * 128] = (
            results[core]["y"].astype(np.float32)
        )
    return out


def kernel(x, w, gamma, beta):
    if not _NC_CACHE:
        _NC_CACHE.append(_build_nc())
    nc = _NC_CACHE[0]
    maps = _prep_inputs(x, w, gamma, beta)
    res = run_bass_kernel_spmd(nc, maps, core_ids=list(range(N_CORES)))
    return _assemble(res.results)
